# revision 11
# baseline (speedup 1.0000x reference)
"""Adaptive embedding (4-cluster masked embedding + projection) on 8 trn2 cores.

Sharding: data-parallel over the batch dim - each of the 8 NeuronCores handles
one batch row (2048 tokens); the embedding/projection tables are replicated.

Host does ROUTING only (cluster assignment, stable sort, padded index arrays);
the device gathers rows from the full tables with indirect DMA, projects
clusters 1-3 on the PE, and writes cluster-sorted output. The host
inverse-permutes rows into token order afterwards.

Perf notes (measured on HW):
- Gather stream: indirect DMA costs ~1.1us fixed per 128-row instruction on
  the GpSimd Q7 (row count barely matters), so the 19 tile-gathers set a
  ~27us floor; the whole compute/store pipeline must hide behind it.
- The PE is the secondary pacer. Transposes are merged: cluster-2 tiles in
  pairs (both land in one [128,128] transpose; projection weights are
  duplicated at partition bases 0/64), cluster-3 tiles in quads (gathers
  fetch 32 contiguous elements per token - its row plus the next row's
  junk - so four 16-row x^T blocks land at partition bases 0/32/64/96;
  junk rows multiply zero weight rows). emb3 is padded by one row so the
  32-element reads of the last vocab row stay in bounds.
- bf16 tables/weights/compute/output halve every byte moved; tolerance is
  2e-2 and bf16 costs ~2.4e-3.
- Output DRAM layout is partition-major [128, ntiles*1024] per cluster so a
  store of k tiles is 128 descriptors of k*2KB (HWDGE, no Q7 involvement).
  The host de-interleaves while inverse-permuting (free).
- The cluster-0 store is emitted LAST on the sync queue: it depends on the
  last gather and the sync engine runs in program order - emitted earlier it
  blocks every chunk store behind it.
- The sqrt(D_PROJ)=32 output scale is an exact power of two, folded into the
  emb0 table and the projection matrices bit-exactly before bf16 rounding.
"""

import numpy as np

CUTOFFS = (0, 20000, 40000, 200000, 267735)
D_PROJ = 1024
DES = (1024, 256, 64, 16)
N_CORES = 8
P = 128

_BUILD_CACHE = {}
LAST_RESULT = None  # BassKernelResults of the most recent run (for profiling)

# store chunk size in tiles: overlap compute with output DMA
STORE_CHUNK = 3
# matmul N per instruction: one PSUM bank (a matmul may not cross banks)
MM_N = 512


def _build(caps, vocab_sizes):
    """Build the SPMD Bass program. caps[i] = 128-token tiles per cluster,
    identical on every core."""
    import concourse.bass as bass
    import concourse.bacc as bacc
    import concourse.tile as tile
    from concourse import mybir

    f32 = mybir.dt.float32
    bf16 = mybir.dt.bfloat16
    i32 = mybir.dt.int32
    nts = list(caps)
    ntsum = sum(nts)
    col0 = [0, nts[0], nts[0] + nts[1], nts[0] + nts[1] + nts[2]]

    nc = bacc.Bacc("TRN2", target_bir_lowering=False)
    emb = [
        nc.dram_tensor(f"emb{i}", [vocab_sizes[i], DES[i]], bf16, kind="ExternalInput")
        for i in range(4)
    ]
    proj1 = nc.dram_tensor("proj1", [DES[1], D_PROJ], bf16, kind="ExternalInput")
    # proj2 duplicated at partition bases 0/64; proj3 at 0/32/64/96 (zeros
    # between) - host-built, so merged transposes can feed tile_position'd
    # matmuls directly
    p2d_in = nc.dram_tensor("p2d", [P, D_PROJ], bf16, kind="ExternalInput")
    p3q_in = nc.dram_tensor("p3q", [P, D_PROJ], bf16, kind="ExternalInput")
    idx_all = nc.dram_tensor("idx_all", [P, ntsum], i32, kind="ExternalInput")
    ident_in = nc.dram_tensor("ident", [P, P], bf16, kind="ExternalInput")
    # partition-major output: row p holds chunks [t*1024:(t+1)*1024] for the
    # token at sorted position t*128+p
    out = [
        nc.dram_tensor(f"out{i}", [P, nts[i] * D_PROJ], bf16, kind="ExternalOutput")
        for i in range(4)
    ]

    with tile.TileContext(nc) as tc:
        with (
            tc.tile_pool(name="const", bufs=1) as cpool,
            tc.tile_pool(name="xt", bufs=6) as xtpool,
            tc.tile_pool(name="tpsum", bufs=2, space="PSUM") as tppool,
            tc.tile_pool(name="mpsum", bufs=3, space="PSUM") as mpool,
        ):
            idxt_all = cpool.tile([P, ntsum], i32, name="idxt_all")
            # SWDGE (same engine as the gathers): the descriptors hit the
            # rings ~1.5us before the HWDGE path gets scheduled at startup
            nc.gpsimd.dma_start(out=idxt_all[:], in_=idx_all[:])
            ident = cpool.tile([P, P], bf16, name="ident")
            nc.sync.dma_start(out=ident[:], in_=ident_in[:])

            # projection weights straight to SBUF in bf16 - no staging casts
            p2d = cpool.tile([P, D_PROJ], bf16, name="p2d")
            nc.sync.dma_start(out=p2d[:], in_=p2d_in[:])
            p1k = []
            for k in range(2):
                t = cpool.tile([P, D_PROJ], bf16, name=f"p1k{k}")
                nc.sync.dma_start(out=t[:], in_=proj1[k * P : (k + 1) * P, :])
                p1k.append(t)
            p3q = cpool.tile([P, D_PROJ], bf16, name="p3q")
            nc.sync.dma_start(out=p3q[:], in_=p3q_in[:])

            # prime the ACT engine's table for f32->bf16 copy before the
            # first PSUM evacuation needs it (the load costs ~1.3us)
            prime_f32 = cpool.tile([1, 16], f32, name="prime_f32")
            nc.gpsimd.memset(prime_f32[:], 0.0)
            prime_out = cpool.tile([1, 16], bf16, name="prime_out")
            nc.scalar.copy(out=prime_out[:], in_=prime_f32[:])

            # gathered-row widths per tile: c3 fetches 32 elements per token
            # (row + following junk row) so tiles sit at 32-column stride
            GW = (1024, 256, 64, 32)
            g = [cpool.tile([P, nts[i] * GW[i]], bf16, name=f"g{i}") for i in range(4)]
            obuf = {
                i: cpool.tile([P, nts[i] * D_PROJ], bf16, name=f"obuf{i}")
                for i in (1, 2, 3)
            }

            def gather_tiles(i, t0, t1):
                # HW indirect DMA consumes one index per partition and copies
                # (out free size) contiguous elements: one instruction per
                # 128-token tile.
                gw = GW[i]
                for t in range(t0, t1):
                    nc.gpsimd.indirect_dma_start(
                        out=g[i][:, t * gw : (t + 1) * gw],
                        out_offset=None,
                        in_=emb[i][:],
                        in_offset=bass.IndirectOffsetOnAxis(
                            ap=idxt_all[:, col0[i] + t : col0[i] + t + 1], axis=0
                        ),
                    )

            # gather schedule: c1 first (PE-heavy, computes during the c2
            # gather stream), c3's cheap tiles late, cluster 0 (copy-only)
            # last so the final tail is just its store
            for i in (1, 2, 3, 0):
                gather_tiles(i, 0, nts[i])

            nhalf = D_PROJ // MM_N

            def evac(i, t, ps):
                ob = obuf[i]
                nc.vector.tensor_copy(
                    out=ob[:, t * D_PROJ : t * D_PROJ + 512], in_=ps[:, 0:512]
                )
                nc.scalar.copy(
                    out=ob[:, t * D_PROJ + 512 : (t + 1) * D_PROJ],
                    in_=ps[:, 512:1024],
                )

            def store_chunk(i, t0, t1):
                nc.sync.dma_start(
                    out=out[i][:, t0 * D_PROJ : t1 * D_PROJ],
                    in_=obuf[i][:, t0 * D_PROJ : t1 * D_PROJ],
                )

            def maybe_store(i, done):
                if done == nts[i] or done % STORE_CHUNK == 0:
                    c0 = ((done - 1) // STORE_CHUNK) * STORE_CHUNK
                    store_chunk(i, c0, done)

            def mm(ps, lhsT, rhs_tile, rbase):
                for n in range(D_PROJ // MM_N):
                    nc.tensor.matmul(
                        ps[:, n * MM_N : (n + 1) * MM_N],
                        lhsT,
                        rhs_tile[rbase : rbase + lhsT.shape[0], n * MM_N : (n + 1) * MM_N],
                        start=True,
                        stop=True,
                    )

            # cluster 1: K=256, two 128-row chunks per tile
            for t in range(nts[1]):
                lhs = []
                for k in range(2):
                    tp = tppool.tile([P, P], bf16, tag="tp", name=f"tp1_{t}_{k}")
                    x = xtpool.tile([P, P], bf16, tag="xt", name=f"xt1_{t}_{k}")
                    lo = t * 256 + k * P
                    nc.tensor.transpose(
                        out=tp[:], in_=g[1][:, lo : lo + P], identity=ident[:]
                    )
                    nc.vector.tensor_copy(out=x[:], in_=tp[:])
                    lhs.append(x)
                ps = mpool.tile([P, D_PROJ], f32, tag="ps", name=f"ps1_{t}")
                for n in range(D_PROJ // MM_N):
                    for k in range(2):
                        nc.tensor.matmul(
                            ps[:, n * MM_N : (n + 1) * MM_N],
                            lhs[k][:],
                            p1k[k][:, n * MM_N : (n + 1) * MM_N],
                            start=(k == 0),
                            stop=(k == 1),
                        )
                evac(1, t, ps)
                maybe_store(1, t + 1)

            # cluster 2: transpose tiles in pairs - both x^T blocks land in
            # one [128,128] PE transpose at partition bases 0 and 64
            for pr in range((nts[2] + 1) // 2):
                t0 = 2 * pr
                two = t0 + 1 < nts[2]
                w = 128 if two else 64
                tp = tppool.tile([P, P], bf16, tag="tp", name=f"tp2_{pr}")
                x = xtpool.tile([P, P], bf16, tag="xt", name=f"xt2_{pr}")
                nc.tensor.transpose(
                    out=tp[:w, :], in_=g[2][:, t0 * 64 : t0 * 64 + w], identity=ident[:]
                )
                nc.vector.tensor_copy(out=x[:w, :], in_=tp[:w, :])
                for t in range(t0, t0 + (2 if two else 1)):
                    base = (t - t0) * 64
                    ps = mpool.tile([P, D_PROJ], f32, tag="ps", name=f"ps2_{t}")
                    mm(ps, x[base : base + 64, :], p2d, base)
                    evac(2, t, ps)
                    maybe_store(2, t + 1)

            # cluster 3: tiles in triples - 16-row x^T blocks at partition
            # bases 0/32/64 from one transpose (SBUF AP base partitions are
            # limited to 0/32/64); junk rows hit zero weights
            for q0 in range(0, nts[3], 3):
                cnt = min(3, nts[3] - q0)
                w = cnt * 32
                tp = tppool.tile([P, P], bf16, tag="tp", name=f"tp3_{q0}")
                x = xtpool.tile([P, P], bf16, tag="xt", name=f"xt3_{q0}")
                nc.tensor.transpose(
                    out=tp[:w, :], in_=g[3][:, q0 * 32 : q0 * 32 + w], identity=ident[:]
                )
                nc.vector.tensor_copy(out=x[:w, :], in_=tp[:w, :])
                for j in range(cnt):
                    t = q0 + j
                    base = j * 32
                    ps = mpool.tile([P, D_PROJ], f32, tag="ps", name=f"ps3_{t}")
                    mm(ps, x[base : base + 16, :], p3q, base)
                    evac(3, t, ps)
            store_chunk(3, 0, nts[3])

            # cluster 0 needs no projection: one flat store, emitted last so
            # it doesn't block the chunk stores in the sync engine's queue
            nc.sync.dma_start(out=out[0][:, :], in_=g[0][:, :])

    nc.compile()
    return nc


def _route(tokens):
    """Cluster assignment, stable sort, local indices, per-cluster caps."""
    toks = np.asarray(tokens).astype(np.int64, copy=False)
    nb, ns = toks.shape
    cuts = np.asarray(CUTOFFS, dtype=np.int64)
    sizes = np.asarray([CUTOFFS[i + 1] - CUTOFFS[i] for i in range(4)], dtype=np.int64)
    cluster = np.searchsorted(cuts[1:-1], toks, side="right")

    orders, counts, locs = [], [], []
    for c in range(nb):
        cl = cluster[c]
        orders.append(np.argsort(cl, kind="stable"))
        counts.append(np.bincount(cl, minlength=4))
        locs.append(np.clip(toks[c] - cuts[cl], 0, sizes[cl] - 1).astype(np.int32))
    counts = np.stack(counts)
    caps = tuple(int(max(1, -(-int(counts[:, i].max()) // P))) for i in range(4))
    return orders, counts, locs, caps


def _idx_cols(counts_c, locs_c, order_c, caps):
    starts = np.concatenate([[0], np.cumsum(counts_c)])
    li = locs_c[order_c]  # local indices, cluster-sorted
    cols = []
    for i in range(4):
        padded = np.zeros(caps[i] * P, np.int32)
        padded[: counts_c[i]] = li[starts[i] : starts[i + 1]]
        # device layout: idx[p, t] = sorted position t*128 + p
        cols.append(padded.reshape(caps[i], P).T)
    return np.ascontiguousarray(np.concatenate(cols, axis=1))


def kernel(tokens, emb0, emb1, emb2, emb3, proj1, proj2, proj3):
    global LAST_RESULT
    import ml_dtypes
    from concourse.bass_utils import run_bass_kernel_spmd

    bf16 = ml_dtypes.bfloat16
    toks = np.asarray(tokens).astype(np.int64, copy=False)
    nb, ns = toks.shape
    assert nb == N_CORES and ns % P == 0

    # sqrt(1024) = 32: exact power of two, folding is bit-exact (also in bf16)
    scale = np.float32(32.0)
    embs = [
        np.ascontiguousarray((np.asarray(e, dtype=np.float32) * s).astype(bf16))
        for e, s in ((emb0, scale), (emb1, 1), (emb2, 1), (emb3, 1))
    ]
    # pad emb3 by one row: cluster-3 gathers read 32 elements (2 rows) per token
    embs[3] = np.ascontiguousarray(
        np.concatenate([embs[3], np.zeros((1, DES[3]), bf16)], axis=0)
    )
    p1 = (np.asarray(proj1, dtype=np.float32) * scale).astype(bf16)
    p2 = (np.asarray(proj2, dtype=np.float32) * scale).astype(bf16)
    p3 = (np.asarray(proj3, dtype=np.float32) * scale).astype(bf16)
    p2d = np.ascontiguousarray(np.concatenate([p2, p2], axis=0))  # [128, 1024]
    p3q = np.zeros((P, D_PROJ), bf16)
    for b in range(3):
        p3q[b * 32 : b * 32 + 16] = p3
    ident = np.eye(P, dtype=np.float32).astype(bf16)

    orders, counts, locs, caps = _route(toks)
    vocab_sizes = tuple(e.shape[0] for e in embs)
    key = (caps, vocab_sizes)
    if key not in _BUILD_CACHE:
        _BUILD_CACHE[key] = _build(caps, vocab_sizes)
    nc = _BUILD_CACHE[key]

    in_maps = []
    for c in range(nb):
        m = {
            "emb0": embs[0],
            "emb1": embs[1],
            "emb2": embs[2],
            "emb3": embs[3],
            "proj1": np.ascontiguousarray(p1),
            "p2d": p2d,
            "p3q": np.ascontiguousarray(p3q),
            "ident": ident,
            "idx_all": _idx_cols(counts[c], locs[c], orders[c], caps),
        }
        in_maps.append(m)

    res = run_bass_kernel_spmd(nc, in_maps, core_ids=list(range(N_CORES)))
    LAST_RESULT = res

    out = np.empty((nb, ns, D_PROJ), np.float32)
    for c in range(nb):
        segs = []
        for i in range(4):
            arr = np.asarray(res.results[c][f"out{i}"])  # [128, caps_i*1024] bf16
            seg = (
                arr.reshape(P, caps[i], D_PROJ)
                .transpose(1, 0, 2)
                .reshape(caps[i] * P, D_PROJ)[: counts[c, i]]
            )
            segs.append(seg)
        out[c][orders[c]] = np.concatenate(segs, axis=0).astype(np.float32)
    return out
